# revision 5
# baseline (speedup 1.0000x reference)
"""Self-contained kernel for nn_ChannelExtensionGlobalAttentionFeatureEncoder.

kernel(**inputs): FULL inputs (left_feat [2,128,96,192] fp32 + params) ->
FULL output [2,128,96,192] fp32.

Sharding: 8 shards = (batch b in {0,1}) x (window-row s in {0..3}); each
NeuronCore owns a 24-row x 192-col slab (one full row of 24x48 attention
windows). Stage 1 (outlook + ffn1 + pos + window-attn + ffn2) runs on a
28-row haloed slab (outlook needs +-2 rows). Host re-shards to the shifted
(rolled) slabs, then stage 2 (shifted window-attn + ffn3) runs. Both stages
are jax.pmap programs over the 8 axon-tunneled trn2 cores; a numpy fallback
guarantees correctness if the device path is unavailable.
"""
import os
import time
import numpy as np

B, C, H, W = 2, 128, 96, 192
K, PAD, NS = 3, 1, 4
WH, WW = H // NS, W // NS          # 24 x 48 windows
SH, SW = WH // 2, WW // 2          # shift 12, 24
TEMP, SCALE = 10000.0, 2.0 * np.pi
HALO = 2

LAST_HW_EXEC_NS = None

# ---------------------------------------------------------------- host helpers

def _pos_np():
    npf = C // 2
    ye = (np.arange(1, H + 1, dtype=np.float64) / (H + 1e-6)) * SCALE
    xe = (np.arange(1, W + 1, dtype=np.float64) / (W + 1e-6)) * SCALE
    dim_t = TEMP ** (2.0 * (np.arange(npf) // 2) / npf)

    def enc(e):
        p = e[:, None] / dim_t
        return np.stack([np.sin(p[:, 0::2]), np.cos(p[:, 1::2])], axis=-1).reshape(
            e.shape[0], npf)

    py, px = enc(ye), enc(xe)
    pos = np.concatenate([
        np.broadcast_to(py[:, None, :], (H, W, npf)),
        np.broadcast_to(px[None, :, :], (H, W, npf))], -1)
    return pos.astype(np.float32)          # [H, W, C]


def _rw_np():
    hid = (np.arange(H) >= (H - WH)).astype(np.int32) + \
          (np.arange(H) >= (H - SH)).astype(np.int32)
    wid = (np.arange(W) >= (W - WW)).astype(np.int32) + \
          (np.arange(W) >= (W - SW)).astype(np.int32)
    region = hid[:, None] * 3 + wid[None, :]
    return region.reshape(NS, WH, NS, WW).transpose(0, 2, 1, 3).reshape(
        NS * NS, WH * WW)                  # [16, 1152]


def _to_np(tree):
    if isinstance(tree, dict):
        return {k: _to_np(v) for k, v in tree.items()}
    return np.asarray(tree, dtype=np.float32)


# ---------------------------------------------------------------- numpy path

def _ln_np(x, g, b):
    m = x.mean(-1, keepdims=True)
    v = x.var(-1, keepdims=True)
    return (x - m) / np.sqrt(v + 1e-5) * g + b


def _sink_np(scores):
    s = scores - scores.max(-2, keepdims=True)
    e = np.exp(s)
    a = e / e.sum(-2, keepdims=True)
    return a / (a.sum(-1, keepdims=True) + 1e-8)


def _gelu_np(x):
    c = np.float32(np.sqrt(2.0 / np.pi))
    return 0.5 * x * (1.0 + np.tanh(c * (x + 0.044715 * x ** 3)))


def _outlook_np(x, p):
    Bn, Cc, Hn, Wn = x.shape
    xh = np.transpose(x, (0, 2, 3, 1))
    xn = _ln_np(xh, p['ln_g'], p['ln_b'])
    v = xn @ p['wv'] + p['bv']
    attn = (xn @ p['wa'] + p['ba']).reshape(Bn, Hn * Wn, K * K, K * K)
    attn = _sink_np(attn)
    vp = np.pad(v, ((0, 0), (PAD, PAD), (PAD, PAD), (0, 0)))
    v_unf = np.stack([vp[:, i:i + Hn, j:j + Wn, :] for i in range(K) for j in range(K)],
                     axis=3).reshape(Bn, Hn * Wn, K * K, Cc)
    out = np.einsum('bnqk,bnkc->bnqc', attn, v_unf).reshape(Bn, Hn, Wn, K, K, Cc)
    acc = np.zeros((Bn, Hn + 2 * PAD, Wn + 2 * PAD, Cc), x.dtype)
    for i in range(K):
        for j in range(K):
            acc[:, i:i + Hn, j:j + Wn, :] += out[:, :, :, i, j, :]
    y = acc[:, PAD:PAD + Hn, PAD:PAD + Wn, :]
    y = y @ p['wo'] + p['bo']
    return np.transpose(y, (0, 3, 1, 2))


def _win_attn_np(x, p, with_shift):
    Bn, Cc, Hn, Wn = x.shape
    xh = np.transpose(x, (0, 2, 3, 1))
    xn = _ln_np(xh, p['ln_g'], p['ln_b'])
    q = xn @ p['wq'] + p['bq']
    k = xn @ p['wk'] + p['bk']
    v = xn @ p['wv'] + p['bv']
    if with_shift:
        q = np.roll(q, (-SH, -SW), axis=(1, 2))
        k = np.roll(k, (-SH, -SW), axis=(1, 2))
        v = np.roll(v, (-SH, -SW), axis=(1, 2))

    def part(t):
        t = t.reshape(Bn, NS, WH, NS, WW, Cc)
        return t.transpose(0, 1, 3, 2, 4, 5).reshape(Bn * NS * NS, WH * WW, Cc)

    qw, kw, vw = part(q), part(k), part(v)
    scores = np.einsum('bnc,bmc->bnm', qw, kw) / float(np.sqrt(Cc))
    if with_shift:
        rw = _rw_np()
        amask = np.where(rw[:, :, None] != rw[:, None, :], -1e9, 0.0).astype(np.float32)
        scores = (scores.reshape(Bn, NS * NS, WH * WW, WH * WW) + amask[None]).reshape(
            Bn * NS * NS, WH * WW, WH * WW)
    attn = _sink_np(scores)
    out = np.einsum('bnm,bmc->bnc', attn, vw)
    out = out.reshape(Bn, NS, NS, WH, WW, Cc).transpose(0, 1, 3, 2, 4, 5).reshape(
        Bn, Hn, Wn, Cc)
    if with_shift:
        out = np.roll(out, (SH, SW), axis=(1, 2))
    out = out @ p['wo'] + p['bo']
    return np.transpose(out, (0, 3, 1, 2))


def _ffn_np(x, p):
    xh = np.transpose(x, (0, 2, 3, 1))
    xn = _ln_np(xh, p['ln_g'], p['ln_b'])
    y = _gelu_np(xn @ p['w1'] + p['b1']) @ p['w2'] + p['b2']
    return np.transpose(y, (0, 3, 1, 2))


def _forward_np(x, params):
    x = x + _outlook_np(x, params['outlook'])
    x = x + _ffn_np(x, params['ffn1'])
    x = x + np.transpose(_pos_np(), (2, 0, 1))[None]
    x = x + _win_attn_np(x, params['win'], False)
    x = x + _ffn_np(x, params['ffn2'])
    x = x + _win_attn_np(x, params['swin'], True)
    x = x + _ffn_np(x, params['ffn3'])
    return x


# ---------------------------------------------------------------- jax path

_JAX_STATE = {}


def _build_jax():
    import jax
    import jax.numpy as jnp

    devs = jax.devices()
    if len(devs) < 8:
        raise RuntimeError(f"need 8 devices, got {len(devs)}")

    def ln(x, g, b):
        m = x.mean(-1, keepdims=True)
        v = x.var(-1, keepdims=True)
        return (x - m) / jnp.sqrt(v + 1e-5) * g + b

    def sink(scores):
        a = jax.nn.softmax(scores, axis=-2)
        return a / (a.sum(-1, keepdims=True) + 1e-8)

    def outlook_slab(xs, rowmask, p):
        # xs [28, 192, C], rowmask [28]
        Hs = H // NS + 2 * HALO
        xn = ln(xs, p['ln_g'], p['ln_b'])
        v = (xn @ p['wv'] + p['bv']) * rowmask[:, None, None]
        attn = (xn @ p['wa'] + p['ba']).reshape(Hs, W, K * K, K * K)
        attn = sink(attn) * rowmask[:, None, None, None]
        attn = attn.reshape(Hs * W, K * K, K * K)
        vp = jnp.pad(v, ((PAD, PAD), (PAD, PAD), (0, 0)))
        v_unf = jnp.stack([vp[i:i + Hs, j:j + W, :] for i in range(K) for j in range(K)],
                          axis=2).reshape(Hs * W, K * K, C)
        out = jnp.einsum('nqk,nkc->nqc', attn, v_unf).reshape(Hs, W, K, K, C)
        acc = jnp.zeros((Hs + 2 * PAD, W + 2 * PAD, C), xs.dtype)
        for i in range(K):
            for j in range(K):
                acc = acc.at[i:i + Hs, j:j + W, :].add(out[:, :, i, j, :])
        y = acc[PAD:PAD + Hs, PAD:PAD + W, :]
        y = y[HALO:HALO + H // NS]
        return y @ p['wo'] + p['bo']       # [24, 192, C]

    def ffn_slab(x, p):
        xn = ln(x, p['ln_g'], p['ln_b'])
        return jax.nn.gelu(xn @ p['w1'] + p['b1']) @ p['w2'] + p['b2']

    def win_slab(x, p, rw):
        # x [24, 192, C]; rw None or [4, 1152] int32 region ids
        xn = ln(x, p['ln_g'], p['ln_b'])
        q = xn @ p['wq'] + p['bq']
        k = xn @ p['wk'] + p['bk']
        v = xn @ p['wv'] + p['bv']

        def part(t):
            return t.reshape(WH, NS, WW, C).transpose(1, 0, 2, 3).reshape(
                NS, WH * WW, C)

        qw, kw, vw = part(q), part(k), part(v)
        scores = jnp.einsum('wnc,wmc->wnm', qw, kw) / float(np.sqrt(C))
        if rw is not None:
            amask = jnp.where(rw[:, :, None] != rw[:, None, :],
                              jnp.float32(-1e9), jnp.float32(0.0))
            scores = scores + amask
        attn = sink(scores)
        out = jnp.einsum('wnm,wmc->wnc', attn, vw)
        out = out.reshape(NS, WH, WW, C).transpose(1, 0, 2, 3).reshape(WH, W, C)
        return out @ p['wo'] + p['bo']

    def stage1(xhalo, rowmask, pos_slab, params):
        x1 = xhalo[HALO:HALO + WH] + outlook_slab(xhalo, rowmask, params['outlook'])
        x2 = x1 + ffn_slab(x1, params['ffn1'])
        x3 = x2 + pos_slab
        x4 = x3 + win_slab(x3, params['win'], None)
        x5 = x4 + ffn_slab(x4, params['ffn2'])
        return x5

    def stage2(x5r, rw, params):
        x6 = x5r + win_slab(x5r, params['swin'], rw)
        x7 = x6 + ffn_slab(x6, params['ffn3'])
        return x7

    pm1 = jax.pmap(stage1, in_axes=(0, 0, 0, None))
    pm2 = jax.pmap(stage2, in_axes=(0, 0, None))
    return jax, jnp, pm1, pm2


def _kernel_jax(left_feat, params):
    global LAST_HW_EXEC_NS
    if 'pm' not in _JAX_STATE:
        _JAX_STATE['pm'] = _build_jax()
    jax, jnp, pm1, pm2 = _JAX_STATE['pm']

    x = left_feat                                        # [2, C, 96, 192]
    xh = np.ascontiguousarray(np.transpose(x, (0, 2, 3, 1)))   # [2, 96, 192, C]

    # ---- stage-1 shards: (b, s) -> 28-row halo slab
    xpad = np.pad(xh, ((0, 0), (HALO, HALO), (0, 0), (0, 0)))  # [2, 100, 192, C]
    halos = np.stack([xpad[b, 24 * s:24 * s + WH + 2 * HALO]
                      for b in range(2) for s in range(NS)])    # [8, 28, 192, C]
    rowmask = np.ones((8, WH + 2 * HALO), np.float32)
    for b in range(2):
        rowmask[b * NS + 0, :HALO] = 0.0
        rowmask[b * NS + NS - 1, -HALO:] = 0.0
    pos = _pos_np()                                            # [96, 192, C]
    pos_slabs = np.stack([pos[24 * s:24 * s + WH]
                          for b in range(2) for s in range(NS)])

    t0 = time.time()
    x5 = pm1(halos, rowmask, pos_slabs, params)                 # [8, 24, 192, C]
    x5 = np.asarray(x5)
    t1 = time.time()

    # ---- host reshard: rolled slabs for the shifted window attention
    x5_full = np.empty((2, H, W, C), np.float32)
    for b in range(2):
        for s in range(NS):
            x5_full[b, 24 * s:24 * s + WH] = x5[b * NS + s]
    x5r_full = np.roll(x5_full, (-SH, -SW), axis=(1, 2))
    x5r = np.stack([x5r_full[b, 24 * s:24 * s + WH]
                    for b in range(2) for s in range(NS)])      # [8, 24, 192, C]
    rw = _rw_np()                                               # [16, 1152]
    rws = np.stack([rw[4 * s:4 * s + 4] for b in range(2) for s in range(NS)])

    t2 = time.time()
    x7 = pm2(x5r, rws, params)                                  # [8, 24, 192, C]
    x7 = np.asarray(x7)
    t3 = time.time()
    LAST_HW_EXEC_NS = int(((t1 - t0) + (t3 - t2)) * 1e9)

    # ---- reassemble + un-roll
    yr = np.empty((2, H, W, C), np.float32)
    for b in range(2):
        for s in range(NS):
            yr[b, 24 * s:24 * s + WH] = x7[b * NS + s]
    y = np.roll(yr, (SH, SW), axis=(1, 2))
    return np.ascontiguousarray(np.transpose(y, (0, 3, 1, 2)))


# ---------------------------------------------------------------- entry point

def kernel(left_feat, params):
    left_feat = np.asarray(left_feat, dtype=np.float32)
    params = _to_np(params)
    try:
        return _kernel_jax(left_feat, params)
    except Exception as e:  # pragma: no cover - device-path failure
        import traceback
        traceback.print_exc()
        print("kernel: jax device path failed (%r); using numpy fallback" % (e,))
        return _forward_np(left_feat, params).astype(np.float32)


# revision 7
# speedup vs baseline: 1.7648x; 1.7648x over previous
"""Self-contained kernel for nn_ChannelExtensionGlobalAttentionFeatureEncoder.

kernel(**inputs): FULL inputs (left_feat [2,128,96,192] fp32 + params) ->
FULL output [2,128,96,192] fp32.

Sharding: 8 shards = (batch b in {0,1}) x (window-row s in {0..3}); each
NeuronCore owns a 24-row x 192-col slab (one full row of 24x48 attention
windows). Stage 1 (outlook + ffn1 + pos + window-attn + ffn2) runs on a
28-row haloed slab (outlook needs +-2 rows). Host re-shards to the shifted
(rolled) slabs, then stage 2 (shifted window-attn + ffn3) runs. Both stages
are jax.pmap programs over the 8 axon-tunneled trn2 cores; a numpy fallback
guarantees correctness if the device path is unavailable.
"""
import os
import time
import numpy as np

B, C, H, W = 2, 128, 96, 192
K, PAD, NS = 3, 1, 4
WH, WW = H // NS, W // NS          # 24 x 48 windows
SH, SW = WH // 2, WW // 2          # shift 12, 24
TEMP, SCALE = 10000.0, 2.0 * np.pi
HALO = 2

LAST_HW_EXEC_NS = None

# ---------------------------------------------------------------- host helpers

def _pos_np():
    npf = C // 2
    ye = (np.arange(1, H + 1, dtype=np.float64) / (H + 1e-6)) * SCALE
    xe = (np.arange(1, W + 1, dtype=np.float64) / (W + 1e-6)) * SCALE
    dim_t = TEMP ** (2.0 * (np.arange(npf) // 2) / npf)

    def enc(e):
        p = e[:, None] / dim_t
        return np.stack([np.sin(p[:, 0::2]), np.cos(p[:, 1::2])], axis=-1).reshape(
            e.shape[0], npf)

    py, px = enc(ye), enc(xe)
    pos = np.concatenate([
        np.broadcast_to(py[:, None, :], (H, W, npf)),
        np.broadcast_to(px[None, :, :], (H, W, npf))], -1)
    return pos.astype(np.float32)          # [H, W, C]


def _rw_np():
    hid = (np.arange(H) >= (H - WH)).astype(np.int32) + \
          (np.arange(H) >= (H - SH)).astype(np.int32)
    wid = (np.arange(W) >= (W - WW)).astype(np.int32) + \
          (np.arange(W) >= (W - SW)).astype(np.int32)
    region = hid[:, None] * 3 + wid[None, :]
    return region.reshape(NS, WH, NS, WW).transpose(0, 2, 1, 3).reshape(
        NS * NS, WH * WW)                  # [16, 1152]


def _to_np(tree):
    if isinstance(tree, dict):
        return {k: _to_np(v) for k, v in tree.items()}
    return np.asarray(tree, dtype=np.float32)


# ---------------------------------------------------------------- numpy path

def _ln_np(x, g, b):
    m = x.mean(-1, keepdims=True)
    v = x.var(-1, keepdims=True)
    return (x - m) / np.sqrt(v + 1e-5) * g + b


def _sink_np(scores):
    s = scores - scores.max(-2, keepdims=True)
    e = np.exp(s)
    a = e / e.sum(-2, keepdims=True)
    return a / (a.sum(-1, keepdims=True) + 1e-8)


def _gelu_np(x):
    c = np.float32(np.sqrt(2.0 / np.pi))
    return 0.5 * x * (1.0 + np.tanh(c * (x + 0.044715 * x ** 3)))


def _outlook_np(x, p):
    Bn, Cc, Hn, Wn = x.shape
    xh = np.transpose(x, (0, 2, 3, 1))
    xn = _ln_np(xh, p['ln_g'], p['ln_b'])
    v = xn @ p['wv'] + p['bv']
    attn = (xn @ p['wa'] + p['ba']).reshape(Bn, Hn * Wn, K * K, K * K)
    attn = _sink_np(attn)
    vp = np.pad(v, ((0, 0), (PAD, PAD), (PAD, PAD), (0, 0)))
    v_unf = np.stack([vp[:, i:i + Hn, j:j + Wn, :] for i in range(K) for j in range(K)],
                     axis=3).reshape(Bn, Hn * Wn, K * K, Cc)
    out = np.einsum('bnqk,bnkc->bnqc', attn, v_unf).reshape(Bn, Hn, Wn, K, K, Cc)
    acc = np.zeros((Bn, Hn + 2 * PAD, Wn + 2 * PAD, Cc), x.dtype)
    for i in range(K):
        for j in range(K):
            acc[:, i:i + Hn, j:j + Wn, :] += out[:, :, :, i, j, :]
    y = acc[:, PAD:PAD + Hn, PAD:PAD + Wn, :]
    y = y @ p['wo'] + p['bo']
    return np.transpose(y, (0, 3, 1, 2))


def _win_attn_np(x, p, with_shift):
    Bn, Cc, Hn, Wn = x.shape
    xh = np.transpose(x, (0, 2, 3, 1))
    xn = _ln_np(xh, p['ln_g'], p['ln_b'])
    q = xn @ p['wq'] + p['bq']
    k = xn @ p['wk'] + p['bk']
    v = xn @ p['wv'] + p['bv']
    if with_shift:
        q = np.roll(q, (-SH, -SW), axis=(1, 2))
        k = np.roll(k, (-SH, -SW), axis=(1, 2))
        v = np.roll(v, (-SH, -SW), axis=(1, 2))

    def part(t):
        t = t.reshape(Bn, NS, WH, NS, WW, Cc)
        return t.transpose(0, 1, 3, 2, 4, 5).reshape(Bn * NS * NS, WH * WW, Cc)

    qw, kw, vw = part(q), part(k), part(v)
    scores = np.einsum('bnc,bmc->bnm', qw, kw) / float(np.sqrt(Cc))
    if with_shift:
        rw = _rw_np()
        amask = np.where(rw[:, :, None] != rw[:, None, :], -1e9, 0.0).astype(np.float32)
        scores = (scores.reshape(Bn, NS * NS, WH * WW, WH * WW) + amask[None]).reshape(
            Bn * NS * NS, WH * WW, WH * WW)
    attn = _sink_np(scores)
    out = np.einsum('bnm,bmc->bnc', attn, vw)
    out = out.reshape(Bn, NS, NS, WH, WW, Cc).transpose(0, 1, 3, 2, 4, 5).reshape(
        Bn, Hn, Wn, Cc)
    if with_shift:
        out = np.roll(out, (SH, SW), axis=(1, 2))
    out = out @ p['wo'] + p['bo']
    return np.transpose(out, (0, 3, 1, 2))


def _ffn_np(x, p):
    xh = np.transpose(x, (0, 2, 3, 1))
    xn = _ln_np(xh, p['ln_g'], p['ln_b'])
    y = _gelu_np(xn @ p['w1'] + p['b1']) @ p['w2'] + p['b2']
    return np.transpose(y, (0, 3, 1, 2))


def _forward_np(x, params):
    x = x + _outlook_np(x, params['outlook'])
    x = x + _ffn_np(x, params['ffn1'])
    x = x + np.transpose(_pos_np(), (2, 0, 1))[None]
    x = x + _win_attn_np(x, params['win'], False)
    x = x + _ffn_np(x, params['ffn2'])
    x = x + _win_attn_np(x, params['swin'], True)
    x = x + _ffn_np(x, params['ffn3'])
    return x


# ---------------------------------------------------------------- jax path

_JAX_STATE = {}


def _build_jax():
    import jax
    import jax.numpy as jnp

    devs = jax.devices()
    if len(devs) < 8:
        raise RuntimeError(f"need 8 devices, got {len(devs)}")

    def ln(x, g, b):
        m = x.mean(-1, keepdims=True)
        v = x.var(-1, keepdims=True)
        return (x - m) / jnp.sqrt(v + 1e-5) * g + b

    def sink(scores):
        a = jax.nn.softmax(scores, axis=-2)
        return a / (a.sum(-1, keepdims=True) + 1e-8)

    def outlook_slab(xs, rowmask, p):
        # xs [28, 192, C], rowmask [28]
        Hs = H // NS + 2 * HALO
        xn = ln(xs, p['ln_g'], p['ln_b'])
        v = (xn @ p['wv'] + p['bv']) * rowmask[:, None, None]
        attn = (xn @ p['wa'] + p['ba']).reshape(Hs, W, K * K, K * K)
        attn = sink(attn) * rowmask[:, None, None, None]
        attn = attn.reshape(Hs * W, K * K, K * K)
        vp = jnp.pad(v, ((PAD, PAD), (PAD, PAD), (0, 0)))
        v_unf = jnp.stack([vp[i:i + Hs, j:j + W, :] for i in range(K) for j in range(K)],
                          axis=2).reshape(Hs * W, K * K, C)
        out = jnp.einsum('nqk,nkc->nqc', attn, v_unf).reshape(Hs, W, K, K, C)
        acc = jnp.zeros((Hs + 2 * PAD, W + 2 * PAD, C), xs.dtype)
        for i in range(K):
            for j in range(K):
                acc = acc.at[i:i + Hs, j:j + W, :].add(out[:, :, i, j, :])
        y = acc[PAD:PAD + Hs, PAD:PAD + W, :]
        y = y[HALO:HALO + H // NS]
        return y @ p['wo'] + p['bo']       # [24, 192, C]

    def ffn_slab(x, p):
        xn = ln(x, p['ln_g'], p['ln_b'])
        return jax.nn.gelu(xn @ p['w1'] + p['b1']) @ p['w2'] + p['b2']

    def win_slab(x, p, rw):
        # x [24, 192, C]; rw None or [4, 1152] int32 region ids
        xn = ln(x, p['ln_g'], p['ln_b'])
        q = xn @ p['wq'] + p['bq']
        k = xn @ p['wk'] + p['bk']
        v = xn @ p['wv'] + p['bv']

        def part(t):
            return t.reshape(WH, NS, WW, C).transpose(1, 0, 2, 3).reshape(
                NS, WH * WW, C)

        qw, kw, vw = part(q), part(k), part(v)
        scores = jnp.einsum('wnc,wmc->wnm', qw, kw) / float(np.sqrt(C))
        if rw is not None:
            amask = jnp.where(rw[:, :, None] != rw[:, None, :],
                              jnp.float32(-1e9), jnp.float32(0.0))
            scores = scores + amask
        attn = sink(scores)
        out = jnp.einsum('wnm,wmc->wnc', attn, vw)
        out = out.reshape(NS, WH, WW, C).transpose(1, 0, 2, 3).reshape(WH, W, C)
        return out @ p['wo'] + p['bo']

    pos_const = jnp.asarray(_pos_np())        # [96, 192, C] baked into the NEFF

    def pos_for(sidx):
        return jax.lax.dynamic_slice(pos_const, (sidx * WH, 0, 0), (WH, W, C))

    def stage1_body(xhalo, rowmask, sidx, params):
        x1 = xhalo[HALO:HALO + WH] + outlook_slab(xhalo, rowmask, params['outlook'])
        x2 = x1 + ffn_slab(x1, params['ffn1'])
        x3 = x2 + pos_for(sidx)
        x4 = x3 + win_slab(x3, params['win'], None)
        x5 = x4 + ffn_slab(x4, params['ffn2'])
        return x5

    def stage2_body(x5r, rw, params):
        x6 = x5r + win_slab(x5r, params['swin'], rw)
        x7 = x6 + ffn_slab(x6, params['ffn3'])
        return x7

    def merged(xhalo, rowmask, sidx, rw, params):
        x5 = stage1_body(xhalo, rowmask, sidx, params)
        # neighbor exchange: device (b,s) receives rows [24(s+1), 24(s+1)+12)
        perm = [(b * NS + (s + 1) % NS, b * NS + s)
                for b in range(2) for s in range(NS)]
        recv = jax.lax.ppermute(x5[:SH], 'i', perm)
        x5r = jnp.roll(jnp.concatenate([x5[SH:], recv], axis=0), -SW, axis=1)
        return stage2_body(x5r, rw, params)

    pm1 = jax.pmap(stage1_body, in_axes=(0, 0, 0, None), axis_name='i')
    pm2 = jax.pmap(stage2_body, in_axes=(0, 0, None), axis_name='i')
    pmm = jax.pmap(merged, in_axes=(0, 0, 0, 0, None), axis_name='i')
    return jax, jnp, pm1, pm2, pmm


def _kernel_jax(left_feat, params):
    global LAST_HW_EXEC_NS
    if 'pm' not in _JAX_STATE:
        _JAX_STATE['pm'] = _build_jax()
    jax, jnp, pm1, pm2, pmm = _JAX_STATE['pm']

    x = left_feat                                        # [2, C, 96, 192]
    xh = np.ascontiguousarray(np.transpose(x, (0, 2, 3, 1)))   # [2, 96, 192, C]

    # ---- stage-1 shards: (b, s) -> 28-row halo slab
    xpad = np.pad(xh, ((0, 0), (HALO, HALO), (0, 0), (0, 0)))  # [2, 100, 192, C]
    halos = np.stack([xpad[b, 24 * s:24 * s + WH + 2 * HALO]
                      for b in range(2) for s in range(NS)])    # [8, 28, 192, C]
    rowmask = np.ones((8, WH + 2 * HALO), np.float32)
    for b in range(2):
        rowmask[b * NS + 0, :HALO] = 0.0
        rowmask[b * NS + NS - 1, -HALO:] = 0.0
    sidx = np.array([s for b in range(2) for s in range(NS)], np.int32)
    rw = _rw_np()                                               # [16, 1152]
    rws = np.stack([rw[4 * s:4 * s + 4] for b in range(2) for s in range(NS)])

    if os.environ.get("KERNEL_TWO_STAGE"):
        t0 = time.time()
        x5 = np.asarray(pm1(halos, rowmask, sidx, params))      # [8, 24, 192, C]
        t1 = time.time()
        x5_full = np.empty((2, H, W, C), np.float32)
        for b in range(2):
            for s in range(NS):
                x5_full[b, 24 * s:24 * s + WH] = x5[b * NS + s]
        x5r_full = np.roll(x5_full, (-SH, -SW), axis=(1, 2))
        x5r = np.stack([x5r_full[b, 24 * s:24 * s + WH]
                        for b in range(2) for s in range(NS)])
        t2 = time.time()
        x7 = np.asarray(pm2(x5r, rws, params))                  # [8, 24, 192, C]
        t3 = time.time()
        LAST_HW_EXEC_NS = int(((t1 - t0) + (t3 - t2)) * 1e9)
    else:
        t0 = time.time()
        x7 = np.asarray(pmm(halos, rowmask, sidx, rws, params))
        t1 = time.time()
        LAST_HW_EXEC_NS = int((t1 - t0) * 1e9)

    # ---- reassemble + un-roll
    yr = np.empty((2, H, W, C), np.float32)
    for b in range(2):
        for s in range(NS):
            yr[b, 24 * s:24 * s + WH] = x7[b * NS + s]
    y = np.roll(yr, (SH, SW), axis=(1, 2))
    return np.ascontiguousarray(np.transpose(y, (0, 3, 1, 2)))


# ---------------------------------------------------------------- entry point

def kernel(left_feat, params):
    left_feat = np.asarray(left_feat, dtype=np.float32)
    params = _to_np(params)
    try:
        return _kernel_jax(left_feat, params)
    except Exception as e:  # pragma: no cover - device-path failure
        import traceback
        traceback.print_exc()
        print("kernel: jax device path failed (%r); using numpy fallback" % (e,))
        return _forward_np(left_feat, params).astype(np.float32)
